# revision 44
# baseline (speedup 1.0000x reference)
"""Trainium2 Bass kernel for nn_AudioNetwork (4-block STFT resonator chain).

Algorithm notes
---------------
Per block: frame x (win 2048, hop 1024), rfft, per-bin linear recurrence over
frames out_i = (spec_i + out_{i-1}) * tc, irfft, hann-windowed overlap-add,
tanh(gain * s).  Since every recurrence step multiplies by tc, bins with
tc == 0 never contribute: the (i)DFT only needs the nonzero bins of tc
(~10 of 1025 for the reference init).  Both transforms become tiny matmuls.

Device layout (per core, 4 batch elements), v4 (ACT-minimal pipeline):
  The signal lives in SBUF as fp16 tiles (128 samples-in-chunk, KT, 1024 cols)
  where col = batch*256 + chunk; host pre-transposes/casts.

  The Activation engine is the bottleneck (tanh of 4 x 1M samples at
  0.833ns/element with no fp16 speedup), so v4 keeps ACT for tanh only:
  all DMAs ride SP/DVE HWDGE rings plus the Pool SWDGE queue, and the
  PSUM->SBUF spectrum staging copies run on Pool.  Tanh is issued per
  (m-pair, group) from a 2-bank PSUM tile (free size 1024 amortizes the
  ~185ns per-call access latency); the inverse matmuls double-buffer
  through 3 such tiles while uv keeps 2 banks (8 banks total).

  Forward: spec_i needs frame i = [chunk_i, chunk_{i+1}] but
  cos/sin(2*pi*k*(s+1024)/2048) = (-1)^k * cos/sin(2*pi*k*s/2048), so only
  the half-window matrix U is computed; the second half is sign * U shifted
  by one frame.  Per group, one Pool copy stages uv (PSUM fp32) into the
  (NF+1)-wide uvs buffer (col NF stays 0 for the edge), one DVE
  scalar_tensor_tensor forms both batches' scan input, and per batch a DVE
  scan (fp32 state) plus a Pool cross-partition copy build the stacked
  [out_cur; out_prev] operand for the inverse.  tr stays fp32 since a 2^-11
  error in tc compounds over 256 frames.

  The inverse DFT matrices are scaled by 2**10 (compensated in the tanh
  scale) so the hann-window tails stay in fp16 normal range.  When the
  mixer weights are all equal (softmax of the zero mixer), the accumulator
  skips the x-copy: the first flush computes acc = x + y1 directly, and
  adds split across DVE (even m) and Pool (odd m) one block late.  The
  final block accumulates per (m-pair, group) and streams the output DMA
  per pair from SP.
"""

import numpy as np
from contextlib import ExitStack, nullcontext as _null

import concourse.bass as bass
import concourse.tile as tile
from concourse import bacc, mybir
from concourse import bass_utils

F32 = mybir.dt.float32
F16 = mybir.dt.float16
WS = 2048
STEP = 1024
NCOEF = WS // 2 + 1
NBLK = 4
B = 32
T = 262144
NCORES = 8
BL = B // NCORES          # batch per core
NF = T // STEP            # 256 frames/chunks
KT = STEP // 128          # 8 K-tiles of the forward contraction
COLS = BL * NF            # 1024 free columns (batch-major)
MAX_BINS_PER_CHUNK = 32   # 2*nb must fit in a 64-row half
WI_SCALE = 1024.0         # keeps hann tails in fp16 normal range
LEAD_NS = 7570            # first-tanh time in the simulated cadence
PERIOD_NS = 9000          # per-block tanh cadence (sim), used as a
                          # scheduling hint only -- never a runtime wait

_CACHE = {}


def _plan_chunks(tc_vec):
    nz = np.nonzero(tc_vec)[0]
    if len(nz) == 0:
        nz = np.array([1], dtype=np.int64)  # dummy bin with tc=0: contributes 0
    chunks = [nz[i:i + MAX_BINS_PER_CHUNK] for i in range(0, len(nz), MAX_BINS_PER_CHUNK)]
    return chunks


def _host_matrices(tc_vec, chunks):
    """Build per-chunk constant arrays (float64 math, fp16/fp32 storage)."""
    hann = 0.5 - 0.5 * np.cos(2.0 * np.pi * np.arange(WS) / WS)
    out = []
    for bins in chunks:
        nb = len(bins)
        k = bins.astype(np.float64)
        tcv = tc_vec[bins].astype(np.float64)
        s = np.arange(STEP, dtype=np.float64)
        ang = 2.0 * np.pi * np.outer(s, k) / WS                      # (1024, nb)
        # duplicated on both column halves: the matmul then writes U to
        # partitions 0:64 and 64:128 at no extra PE cost, keeping both scans
        # partition-aligned
        bf = np.zeros((STEP, 128))
        bf[:, 0:nb] = np.cos(ang) * tcv
        bf[:, nb:2 * nb] = -np.sin(ang) * tcv
        bf[:, 64:64 + 2 * nb] = bf[:, 0:2 * nb]
        bf_t = bf.reshape(KT, 128, 128).transpose(1, 0, 2)           # (128, 8, 128)
        sign = np.zeros((128, 1))
        sign[0:nb, 0] = (-1.0) ** k
        sign[nb:2 * nb, 0] = (-1.0) ** k
        sign[64:64 + 2 * nb] = sign[0:2 * nb]
        tcrep = np.zeros((128, NF))
        tcrep[0:nb] = tcv[:, None]
        tcrep[nb:2 * nb] = tcv[:, None]
        tcrep[64:64 + 2 * nb] = tcrep[0:2 * nb]
        w = np.where((bins == 0) | (bins == WS // 2), 1.0, 2.0)
        s2 = np.arange(WS, dtype=np.float64)
        ang2 = 2.0 * np.pi * np.outer(k, s2) / WS                    # (nb, 2048)
        are = (w[:, None] / WS) * np.cos(ang2) * hann * WI_SCALE
        aim = -(w[:, None] / WS) * np.sin(ang2) * hann * WI_SCALE
        w1 = np.concatenate([are[:, :STEP], aim[:, :STEP]], axis=0)  # cur frame
        w2 = np.concatenate([are[:, STEP:], aim[:, STEP:]], axis=0)  # prev frame
        pad = np.zeros((64 - 2 * nb, WS // 2))
        winv = np.concatenate([w1, pad, w2, pad], axis=0).reshape(128, KT, 128)
        out.append(dict(
            nb=nb,
            bf=np.ascontiguousarray(bf_t, dtype=np.float16),
            winv=np.ascontiguousarray(winv, dtype=np.float16),
            sign=np.ascontiguousarray(sign, dtype=np.float16),
            tcrep=np.ascontiguousarray(tcrep, dtype=np.float32),
        ))
    return out


def _build(chunk_sizes, gains, wmix):
    """Trace+compile the Bass program. chunk_sizes: tuple of tuples of nb per block."""
    nc = bacc.Bacc("TRN2", target_bir_lowering=False, debug=False)
    # x arrives host-pre-transposed to the device layout (sample-in-chunk on
    # partitions, k-tile, batch*frame columns) in fp16; the output is the
    # fp16 accumulator in the same layout, un-transposed and scaled on host
    x_d = nc.dram_tensor("x", (128, KT, COLS), F16, kind="ExternalInput").ap()
    out_d = nc.dram_tensor("out", (128, KT, COLS), F16, kind="ExternalOutput").ap()
    # per-chunk constants, chunk-major so each chunk's (bf, wi) is one
    # contiguous 790ns DMA that can be scheduled exactly when needed
    chunks_flat = [(kb, c) for kb in range(NBLK) for c in range(len(chunk_sizes[kb]))]
    nch_tot = len(chunks_flat)
    cons = {
        "bfA": nc.dram_tensor("bfA", (nch_tot, 128, KT, 128), F16, kind="ExternalInput").ap(),
        "wiA": nc.dram_tensor("wiA", (nch_tot, 128, KT, 128), F16, kind="ExternalInput").ap(),
        "sgA": nc.dram_tensor("sgA", (128, nch_tot), F16, kind="ExternalInput").ap(),
        "trA": nc.dram_tensor("trA", (128, NF * nch_tot), F32, kind="ExternalInput").ap(),
    }

    mult = mybir.AluOpType.mult
    add = mybir.AluOpType.add
    bypass_op = mybir.AluOpType.bypass
    Tanh = mybir.ActivationFunctionType.Tanh

    w_equal = bool(np.allclose(wmix, wmix[0], rtol=1e-7, atol=0.0))

    with tile.TileContext(nc) as tc, ExitStack() as ctx:
        cpool = ctx.enter_context(tc.tile_pool(name="const", bufs=1))
        big = ctx.enter_context(tc.tile_pool(name="big", bufs=1))
        work = ctx.enter_context(tc.tile_pool(name="work", bufs=2))
        # PSUM budget (8 banks): uv (1 bank) x2 + ip2 (2 banks) x3 = 8
        pmm = ctx.enter_context(tc.tile_pool(name="pmm", bufs=2, space="PSUM"))
        pmi = ctx.enter_context(tc.tile_pool(name="pmi", bufs=3, space="PSUM"))

        bf_t = [cpool.tile([128, KT, 128], F16, tag=f"bf{i}", name=f"bf_t{i}")
                for i in range(nch_tot)]
        wi_t = [cpool.tile([128, KT, 128], F16, tag=f"wi{i}", name=f"wi_t{i}")
                for i in range(nch_tot)]
        sgA_t = cpool.tile([128, nch_tot], F16, tag="sgA", name="sgA_t")
        trA_t = cpool.tile([128, NF * nch_tot], F32, tag="trA", name="trA_t")

        def bf_ap(kb, c):
            return bf_t[chunks_flat.index((kb, c))][:]

        def wi_ap(kb, c):
            return wi_t[chunks_flat.index((kb, c))][:]

        xbuf = [big.tile([128, KT, COLS], F16, tag=f"xb{i}", name=f"xb{i}") for i in range(2)]
        accb = big.tile([128, KT, COLS], F16, tag="acc", name="acc")
        # scat col k: rows 0:64 = out_{k-1} (scan), rows 64:128 = out_{k-2}
        # (cross-partition shifted copy); col 0 zero feeds the overlap edge.
        # Double-buffered per block so the next block's scans never wait for
        # this block's inverse matmuls to finish reading.
        scats = [cpool.tile([128, BL, NF + 1], F16, tag=f"scat{i}", name=f"scat{i}")
                 for i in range(2)]
        for s in scats:
            nc.vector.memset(s[:, :, 0:2], 0.0)
        # uvs col NF stays zero: the sign-combine then covers all 256 cols
        uvs = cpool.tile([128, BL, NF + 1], F16, tag="uvs", name="uvs")
        nc.vector.memset(uvs[:, :, NF:NF + 1], 0.0)
        # the add-fence reads these columns before the first real write
        nc.vector.memset(accb[:, :, 0:1], 0.0)

        # ---- DMA plan: ACT keeps only its auto-inserted act-table load and
        # two early x slabs (it idles until the first tanh anyway); SP + DVE
        # rings carry the rest of the hot loads and the output stream; the
        # Pool SWDGE queue takes the latency-tolerant constants ----
        # DMA plan: x rides SP + ACT's HWDGE rings as half-slabs with ALL
        # group-0 halves first, so block 0's forward g0 (the head of the
        # dependency chain) is fed ~2us earlier; wi0 lands before the g1
        # halves (the inverse needs it first); the per-block bf/wi pairs
        # stream on SP afterwards, each arriving well before its block.
        # Pool's SWDGE queue takes the small early constants.
        nc.sync.dma_start(bf_t[0][:], cons["bfA"][0])
        for g in range(2):
            for m in range(KT):
                eng = nc.sync if m % 2 == 0 else nc.scalar
                eng.dma_start(xbuf[0][:, m, g * 512:(g + 1) * 512],
                              x_d[:, m, g * 512:(g + 1) * 512])
            if g == 0:
                nc.sync.dma_start(wi_t[0][:], cons["wiA"][0])
        for i in range(1, nch_tot):
            nc.sync.dma_start(bf_t[i][:], cons["bfA"][i])
            nc.sync.dma_start(wi_t[i][:], cons["wiA"][i])
        nc.gpsimd.dma_start(sgA_t[:], cons["sgA"][:])
        nc.gpsimd.dma_start(trA_t[:, 0:NF], cons["trA"][:, 0:NF])
        if nch_tot > 1:
            nc.sync.dma_start(trA_t[:, NF:], cons["trA"][:, NF:])

        if not w_equal:
            # general-mixer path: acc starts as w0 * x
            for m in range(KT):
                eng = nc.vector if m % 2 == 0 else nc.gpsimd
                eng.tensor_scalar_mul(accb[:, m, :], xbuf[0][:, m, :], float(wmix[0]))

        def acc_update(t, mpair, gcols, eng, first, w):
            if w_equal and first:
                # acc = x + y1: skips a separate accumulator-init copy
                eng.tensor_tensor(accb[:, mpair, gcols], xbuf[0][:, mpair, gcols],
                                  t[:, mpair, gcols], op=add)
            elif w_equal:
                eng.tensor_tensor(accb[:, mpair, gcols], accb[:, mpair, gcols],
                                  t[:, mpair, gcols], op=add)
            else:
                eng.scalar_tensor_tensor(
                    accb[:, mpair, gcols], t[:, mpair, gcols], float(w),
                    accb[:, mpair, gcols], op0=mult, op1=add)



        # ---- block chain ----
        for kb in range(NBLK):
            src = xbuf[kb % 2]
            dst = xbuf[(kb + 1) % 2]
            sizes = chunk_sizes[kb]
            nch = len(sizes)
            inv_sb = None
            if nch > 1:
                inv_sb = big.tile([128, KT, COLS], F32, tag="is", name=f"is{kb}")
            for c, nb in enumerate(sizes):
                ci = chunks_flat.index((kb, c))
                scat = scats[ci % 2]
                bf = bf_ap(kb, c)
                wi = wi_ap(kb, c)
                sg = sgA_t[:, ci:ci + 1]
                tr = trA_t[:, NF * ci:NF * (ci + 1)]
                gain = float(gains[kb]) / WI_SCALE

                # two independent column-group streams (batches 0-1 / 2-3):
                # PE runs fwd g0, fwd g1, inv g0, inv g1 back to back; the
                # recurrence for each group overlaps the PE work of the other,
                # and the next block's fwd g0 only waits on this block's g0
                # tanh halves
                for g in range(2):
                    uv = pmm.tile([128, 2, NF], F32, tag="uv", name=f"uv{kb}_{c}_{g}")
                    for a in range(KT):
                        nc.tensor.matmul(uv[:], bf[:, a, :],
                                         src[:, a, g * 512:(g + 1) * 512],
                                         start=(a == 0), stop=(a == KT - 1))
                    # DVE stages uv (PSUM fp32 -> fp16; gpsimd cannot touch
                    # PSUM and s2s2d2 ops cannot read two PSUM operands) into
                    # the (NF+1)-wide uvs buffer and forms both batches' scan
                    # inputs in one op; per batch a fp32-state scan, then Pool
                    # shifts rows 0:64 into 64:128 one column later for the
                    # inverse stack
                    nc.vector.tensor_copy(uvs[:, 2 * g:2 * g + 2, 0:NF],
                                          uv[:, :, :])
                    in1 = work.tile([128, 2, NF], F16, tag="in1",
                                    name=f"in1_{kb}_{c}_{g}")
                    nc.vector.scalar_tensor_tensor(
                        in1[:], uvs[:, 2 * g:2 * g + 2, 1:NF + 1], sg,
                        uvs[:, 2 * g:2 * g + 2, 0:NF], op0=mult, op1=add)
                    for bb in range(2):
                        b = 2 * g + bb
                        nc.vector.tensor_tensor_scan(
                            scat[0:64, b, 1:NF + 1], tr[0:64, :], in1[0:64, bb, :],
                            initial=0.0, op0=mult, op1=add)
                        nc.gpsimd.tensor_copy(scat[64:128, b, 1:NF + 1],
                                              scat[0:64, b, 0:NF])
                last = kb == NBLK - 1 and c == nch - 1

                for g in range(2):
                    gcols = slice(g * 512, (g + 1) * 512)
                    for j in range(KT // 2):
                        ip = pmi.tile([128, 2, 512], F32, tag="ips",
                                      name=f"ip{kb}_{c}_{g}_{j}")
                        for jj in range(2):
                            if j == 0:
                                # first pair per group: split the moving
                                # operand per batch so the matmul can start
                                # on batch 0's scan stack while batch 1's is
                                # still being shifted -- shortens the chain
                                # feeding the block's first tanh
                                for bb in range(2):
                                    nc.tensor.matmul(
                                        ip[:, jj, 256 * bb:256 * (bb + 1)],
                                        wi[:, 2 * j + jj, :],
                                        scat[:, 2 * g + bb, 1:NF + 1],
                                        start=True, stop=True)
                            else:
                                nc.tensor.matmul(ip[:, jj, :], wi[:, 2 * j + jj, :],
                                                 scat[:, 2 * g:2 * g + 2, 1:NF + 1],
                                                 start=True, stop=True)
                        mpair = slice(2 * j, 2 * j + 2)
                        if nch == 1:
                            nc.scalar.activation(dst[:, mpair, gcols], ip[:],
                                                 Tanh, scale=gain)
                        else:
                            for jj in range(2):
                                half = inv_sb[:, 2 * j + jj, gcols]
                                if c == 0:
                                    nc.vector.tensor_copy(half, ip[:, jj, :])
                                else:
                                    nc.vector.tensor_add(half, half, ip[:, jj, :])
                            if c == nch - 1:
                                nc.scalar.activation(dst[:, mpair, gcols],
                                                     inv_sb[:, mpair, gcols],
                                                     Tanh, scale=gain)
                        if c == nch - 1:
                            if last:
                                # final block: prompt adds (they gate the
                                # stores); stores stream per (pair, group)
                                # half.  The very last pair splits per-m so
                                # its tail is one small add plus one small
                                # store on ACT's ring, free after its final
                                # tanh.
                                if j == KT // 2 - 1 and g == 1:
                                    acc_update(dst, slice(2 * j, 2 * j + 1),
                                               gcols, nc.gpsimd, kb == 0,
                                               wmix[kb + 1])
                                    acc_update(dst, slice(2 * j + 1, 2 * j + 2),
                                               gcols, nc.vector, kb == 0,
                                               wmix[kb + 1])
                                    nc.sync.dma_start(
                                        out_d[:, 2 * j, gcols],
                                        accb[:, 2 * j, gcols])
                                    nc.scalar.dma_start(
                                        out_d[:, 2 * j + 1, gcols],
                                        accb[:, 2 * j + 1, gcols])
                                else:
                                    acc_update(dst, mpair, gcols, nc.vector,
                                               kb == 0, wmix[kb + 1])
                                    nc.sync.dma_start(out_d[:, mpair, gcols],
                                                      accb[:, mpair, gcols])
                            else:
                                # The list scheduler's internal timing is
                                # optimistic about tanh completion, so it
                                # orders these adds ahead of critical chain
                                # ops.  Give each add a wait timestamp equal
                                # to its own natural ready time (the end of
                                # the tanh feeding it, per the simulated
                                # cadence): behavior is unchanged at runtime,
                                # but the internal model now sees the true
                                # ready order and keeps adds out of the
                                # combine chains.
                                eng = nc.vector if g == 0 else nc.gpsimd
                                t_ready = (LEAD_NS + PERIOD_NS * kb
                                           + (4 * g + j + 1) * 1038 + 200)
                                with tc.tile_wait_until(t_ready * 1e-6):
                                    acc_update(dst, mpair, gcols, eng,
                                               kb == 0, wmix[kb + 1])

    nc.compile()
    return nc


def _const_map(transfers, plans):
    bfs, wis, sgs, trs = [], [], [], []
    for kb in range(NBLK):
        mats = _host_matrices(transfers[kb].astype(np.float64), plans[kb])
        for md in mats:
            bfs.append(md["bf"])
            wis.append(md["winv"])
            sgs.append(md["sign"])
            trs.append(md["tcrep"])
    return {
        "bfA": np.ascontiguousarray(np.stack(bfs, axis=0)),
        "wiA": np.ascontiguousarray(np.stack(wis, axis=0)),
        "sgA": np.ascontiguousarray(np.concatenate(sgs, axis=1)),
        "trA": np.ascontiguousarray(np.concatenate(trs, axis=1)),
    }


def _in_maps(x, const_map):
    # host-side shard + transpose to the device layout:
    # x16t[p, m, b*NF + f] = x[b, f*1024 + m*128 + p]
    xr = np.asarray(x, dtype=np.float32).reshape(B, T).astype(np.float16)
    maps = []
    for core in range(NCORES):
        xc = xr[core * BL:(core + 1) * BL].reshape(BL, NF, KT, 128)
        m = dict(const_map)
        m["x"] = np.ascontiguousarray(xc.transpose(3, 2, 0, 1).reshape(128, KT, COLS))
        maps.append(m)
    return maps


def _out_gather(res, scale):
    outs = []
    for i in range(NCORES):
        ot = res.results[i]["out"].reshape(128, KT, BL, NF)
        oc = ot.transpose(2, 3, 1, 0).reshape(BL, 1, T)
        outs.append(oc)
    return (np.concatenate(outs, axis=0).astype(np.float32) * np.float32(scale))


def kernel(x, transfers, gains, mixer):
    transfers = np.asarray(transfers, dtype=np.float32)
    gains = np.asarray(gains, dtype=np.float64)
    mixer = np.asarray(mixer, dtype=np.float64)
    wm = np.exp(mixer - mixer.max())
    wm = wm / wm.sum()

    plans = [_plan_chunks(transfers[kb]) for kb in range(NBLK)]
    chunk_sizes = tuple(tuple(len(ch) for ch in pl) for pl in plans)
    key = (chunk_sizes, tuple(np.round(gains, 9)), tuple(np.round(wm, 9)))
    if key not in _CACHE:
        _CACHE[key] = _build(chunk_sizes, gains, wm)
    nc = _CACHE[key]

    in_maps = _in_maps(x, _const_map(transfers, plans))
    res = bass_utils.run_bass_kernel_spmd(nc, in_maps, core_ids=list(range(NCORES)))
    w_equal = bool(np.allclose(wm, wm[0], rtol=1e-7, atol=0.0))
    return _out_gather(res, wm[0] if w_equal else 1.0)
